# revision 29
# baseline (speedup 1.0000x reference)
"""AncProbsLayer Trainium2 kernel (8 NeuronCores, SPMD data-parallel over batch b).

Math: for each (m, b, k):  P = expm(tau[m,b] * Q[m,k])  (20x20 GTR rate matrix),
then anc[m,b,l,k,:] = P[m,b,k, seq[m,b,l], :].

Host does only the O(m*k*S^3) eigensolve preprocessing of the 16 tiny 20x20
matrices (R/p/Q/B/eigh -> V, W, lam tables; no b or L dependence).  The device
computes everything that scales with b/L: tau gather + softplus, e=exp(tau*lam)
(ACT), P = (V.e) @ W via PE matmuls, the one-hot gather matmul, and the 21MB
output DMA.  Batch b is sharded 8 ways (4 b's per core, both m on every core).

Per-core device layout:
  pairs pr = m*4+q (q = local b index), halves h in {0,1} = k groups of 4.
  K-dim of the P-matmul is (kq, s) = 80;  output free dim is (k, j) = 160.
  seq one-hot is built as [80part = 4 pairs x 20i, 512l] with host-side +20q
  offsets, compared against the partition index via tensor_scalar(is_equal).
  Gather matmul: out[128l, 160kj] = onehot[20i,128l]^T @ Ptab[20i,160kj].
"""

import sys
import numpy as np

for _p in ("/opt/trn_rl_repo", "/root/.axon_site/_ro/trn_rl_repo"):
    if _p not in sys.path:
        sys.path.append(_p)

import ml_dtypes

M, B, L, K, S = 2, 32, 512, 8, 20
NCORES = 8
BLOC = B // NCORES          # 4 b's per core
NPAIR = M * BLOC            # 8 (m, q) pairs per core
NH = 2                      # k halves
KH = K // NH                # 4 k per half
KD = KH * S                 # 80 = contraction dim per half
KJ = K * S                  # 160 = (k, j) output free dim
NCHUNK = 4                  # L chunks of 128
LC = L // NCHUNK            # 128
EPS = 1e-16

# Matmul compute dtype: fp16 moving operand = 1 cycle/row on PE (fp32 = 4)
# with 10 mantissa bits (rel eps ~5e-4; bf16's 7 bits was ~9e-3 end-to-end).
_MM_NP = "float16"

_GRAPH_CACHE = {}


def _softplus(x):
    return np.log1p(np.exp(-np.abs(x))) + np.maximum(x, 0.0)


def _host_prep(sequences, rate_indices, tau_kernel, exchangeability_kernel,
               equilibrium_kernel):
    """Eigensolve preprocessing of the 16 20x20 kernels + input staging."""
    ex = np.asarray(exchangeability_kernel, np.float64)
    eq = np.asarray(equilibrium_kernel, np.float64)
    R = _softplus(0.5 * (ex + np.swapaxes(ex, -1, -2)))          # (m,k,S,S)
    z = eq - eq.max(-1, keepdims=True)
    p = np.exp(z)
    p /= p.sum(-1, keepdims=True)                                # (m,k,S)
    Q = R * p[..., None, :]
    Q = Q - Q.sum(-1, keepdims=True) * np.eye(S)
    mue = -np.sum(p * np.diagonal(Q, axis1=-2, axis2=-1), axis=-1, keepdims=True)
    Q = Q / np.maximum(mue, EPS)[..., None]
    sqrtp = np.sqrt(p)
    Bm = sqrtp[..., :, None] * Q / sqrtp[..., None, :]
    Bm = 0.5 * (Bm + np.swapaxes(Bm, -1, -2))
    lam, U = np.linalg.eigh(Bm)                                  # (m,k,S),(m,k,S,S)
    V = U / sqrtp[..., :, None]                                  # V[m,k,i,s]
    Wm = U * sqrtp[..., :, None]                                 # W[m,k,j,s]

    p_dt = np.dtype(_MM_NP)
    V_in = np.zeros((M * NH, KD, S), p_dt)
    W_in = np.zeros((M * NH, KD, KJ), p_dt)
    lam_in = np.zeros((M * NH, KD, 1), np.float32)
    for m in range(M):
        for h in range(NH):
            mh = m * NH + h
            for kq in range(KH):
                k = h * KH + kq
                r0 = kq * S
                V_in[mh, r0:r0 + S, :] = V[m, k].T.astype(p_dt)      # [s,i]
                W_in[mh, r0:r0 + S, k * S:(k + 1) * S] = Wm[m, k].T.astype(p_dt)
                lam_in[mh, r0:r0 + S, 0] = lam[m, k]

    tau_kT = np.ascontiguousarray(np.asarray(tau_kernel, np.float32).T)  # (32,2)

    seq = np.asarray(sequences)
    ri = np.asarray(rate_indices)
    in_maps = []
    for c in range(NCORES):
        b0 = c * BLOC
        seqo = np.empty((NPAIR, L), np.float32)
        for m in range(M):
            for q in range(BLOC):
                seqo[m * BLOC + q] = seq[m, b0 + q].astype(np.float32) \
                    + 32.0 * (q % 2)
        in_maps.append({
            "seqo": seqo,
            "ri": ri[:, b0:b0 + BLOC].astype(np.float32),
            "taukT": tau_kT,
            "V": V_in,
            "W": W_in,
            "lam": lam_in,
        })
    return in_maps


def _build_graph():
    if "nc" in _GRAPH_CACHE:
        return _GRAPH_CACHE["nc"]
    from contextlib import ExitStack
    import concourse.bass as bass
    import concourse.mybir as mybir
    import concourse.tile as tile
    from concourse import bacc

    f32 = mybir.dt.float32
    mm_dt = getattr(mybir.dt, _MM_NP)
    p_dt = mm_dt
    AF = mybir.ActivationFunctionType
    ALU = mybir.AluOpType

    nc = bacc.Bacc("TRN2", target_bir_lowering=False, debug=False,
                   enable_asserts=False)
    seq_e = nc.declare_dram_parameter("seqo", [NPAIR, L], f32, isOutput=False)
    ri_e = nc.declare_dram_parameter("ri", [M, BLOC], f32, isOutput=False)
    tkt_e = nc.declare_dram_parameter("taukT", [B, M], f32, isOutput=False)
    V_e = nc.declare_dram_parameter("V", [M * NH, KD, S], p_dt, isOutput=False)
    W_e = nc.declare_dram_parameter("W", [M * NH, KD, KJ], p_dt, isOutput=False)
    lam_e = nc.declare_dram_parameter("lam", [M * NH, KD, 1], f32, isOutput=False)
    out_e = nc.declare_dram_parameter("out", [NPAIR, L, KJ], f32, isOutput=True)

    with tile.TileContext(nc) as tc, ExitStack() as ctx:
        const = ctx.enter_context(tc.tile_pool(name="const", bufs=1))
        work = ctx.enter_context(tc.tile_pool(name="work", bufs=3))
        outp = ctx.enter_context(tc.tile_pool(name="outp", bufs=4))
        ps_s = ctx.enter_context(tc.tile_pool(name="ps_s", bufs=1, space="PSUM"))
        ps_p = ctx.enter_context(tc.tile_pool(name="ps_p", bufs=2, space="PSUM"))
        ps_g = ctx.enter_context(tc.tile_pool(name="ps_g", bufs=4, space="PSUM"))

        # ---- static loads + constants
        taukT = const.tile([B, M], f32, tag="taukT")
        nc.sync.dma_start(taukT[:], tkt_e[:])
        ones80 = const.tile([1, KD], f32, tag="ones80")
        nc.vector.memset(ones80[:], 1.0)
        iota32 = const.tile([B, BLOC], f32, tag="iota32")
        nc.gpsimd.iota(iota32[:], pattern=[[0, BLOC]], base=0,
                       channel_multiplier=1,
                       allow_small_or_imprecise_dtypes=True)
        iota64 = const.tile([64, L], f32, tag="iota64")
        nc.gpsimd.iota(iota64[:], pattern=[[0, L]], base=0,
                       channel_multiplier=1,
                       allow_small_or_imprecise_dtypes=True)

        V_t, W_t, lam_t = [], [], []
        for mh in range(M * NH):
            vt = const.tile([KD, S], p_dt, tag=f"V{mh}")
            nc.sync.dma_start(vt[:], V_e[mh])
            wt = const.tile([KD, KJ], p_dt, tag=f"W{mh}")
            nc.sync.dma_start(wt[:], W_e[mh])
            lt = const.tile([KD, 1], f32, tag=f"lam{mh}")
            nc.sync.dma_start(lt[:], lam_e[mh])
            V_t.append(vt); W_t.append(wt); lam_t.append(lt)

        # One-hot: 2 pairs per [64, L] tile at 32-partition boundaries (host
        # adds +32*(q%2) to seq); pad rows 32u+20..32u+31 hold values below
        # 32u+20 so they compare to 0.  Group g = m*2 + q//2.
        seq_t, oh_t = [], []
        for g in range(NPAIR // 2):
            st = const.tile([64, L], f32, tag=f"seq{g}", name=f"seq{g}")
            for u in range(2):
                pr = 2 * g + u
                nc.sync.dma_start(st[32 * u:32 * (u + 1), :],
                                  seq_e[pr, :].partition_broadcast(32))
            oh = const.tile([64, L], mm_dt, tag=f"oh{g}", name=f"oh{g}")
            nc.vector.tensor_tensor(oh[:], st[:], iota64[:], ALU.is_equal)
            seq_t.append(st); oh_t.append(oh)

        # ---- tau gather + softplus + e = exp(tau*lam)
        e_t = []
        for m in range(M):
            ri_bc = work.tile([B, BLOC], f32, tag="ri_bc")
            nc.sync.dma_start(ri_bc[:], ri_e[m, :].partition_broadcast(B))
            ohri = work.tile([B, BLOC], f32, tag="ohri")
            nc.vector.tensor_tensor(ohri[:], ri_bc[:], iota32[:], ALU.is_equal)
            tau_ps = ps_s.tile([1, BLOC], f32, tag="tau_ps")
            nc.tensor.matmul(tau_ps[:], taukT[:, m:m + 1], ohri[:])
            tau_ex = work.tile([1, BLOC], f32, tag="tau_ex")
            nc.scalar.activation(tau_ex[:], tau_ps[:], AF.Exp)
            tau_sb = work.tile([1, BLOC], f32, tag="tau_sb")
            nc.scalar.activation(tau_sb[:], tau_ex[:], AF.Ln, bias=1.0)
            taub_ps = ps_s.tile([KD, BLOC], f32, tag="taub_ps")
            nc.tensor.matmul(taub_ps[:], ones80[:], tau_sb[:])
            for h in range(NH):
                mh = m * NH + h
                et = const.tile([KD, BLOC], f32, tag=f"e{mh}")
                nc.scalar.activation(et[:], taub_ps[:], AF.Exp,
                                     scale=lam_t[mh][:])
                e_t.append(et)

        # ---- per (m, q) pair: P construction + gather + store.
        # P for the 2 pairs of one group live at 32-partition offsets of
        # shared [64, KJ] PSUM/SBUF tiles, so the gather matmul's stationary
        # (onehot slice) and moving (ptab slice) bases match.
        for g in range(NPAIR // 2):
            m = g // 2
            P_ps = ps_p.tile([64, KJ], f32, tag="P_ps", name=f"P_psg{g}")
            ptab = work.tile([64, KJ], mm_dt, tag="ptab", name=f"ptabg{g}")
            for u in range(2):
                pr = 2 * g + u
                q = pr % BLOC
                pslice = slice(32 * u, 32 * u + S)
                for h in range(NH):
                    mh = m * NH + h
                    ve = work.tile([KD, S], p_dt, tag="ve", name="ve")
                    nc.scalar.activation(ve[:], V_t[mh][:], AF.Copy,
                                         scale=e_t[mh][:, q:q + 1])
                    nc.tensor.matmul(P_ps[pslice, :], ve[:], W_t[mh][:],
                                     start=(h == 0), stop=(h == NH - 1))
                nc.scalar.activation(ptab[pslice, :], P_ps[pslice, :],
                                     AF.Copy)
            for u in range(2):
                pr = 2 * g + u
                pslice = slice(32 * u, 32 * u + S)
                for cj in range(NCHUNK // 2):
                    g_ps = ps_g.tile([LC, 2 * KJ], f32, tag="g_ps",
                                     name="g_ps")
                    for w in range(2):
                        ci = 2 * cj + w
                        nc.tensor.matmul(
                            g_ps[:, w * KJ:(w + 1) * KJ],
                            oh_t[g][pslice, LC * ci:LC * (ci + 1)],
                            ptab[pslice, :])
                    o_sb = outp.tile([LC, 2 * KJ], f32, tag="o_sb",
                                     name="o_sb")
                    if (u + cj) % 2 == 0:
                        nc.vector.tensor_copy(o_sb[:], g_ps[:])
                    else:
                        nc.scalar.activation(o_sb[:], g_ps[:], AF.Copy)
                    for w in range(2):
                        ci = 2 * cj + w
                        eng = nc.sync if w == 0 else nc.scalar
                        eng.dma_start(out_e[pr, LC * ci:LC * (ci + 1), :],
                                      o_sb[:, w * KJ:(w + 1) * KJ])

    nc.compile()
    _GRAPH_CACHE["nc"] = nc
    return nc


def _run(inputs, trace=False):
    from concourse.bass_utils import run_bass_kernel_spmd
    in_maps = _host_prep(**inputs)
    nc = _build_graph()
    res = run_bass_kernel_spmd(nc, in_maps, core_ids=list(range(NCORES)),
                               trace=trace)
    full = np.empty((M, B, L, K, S), np.float32)
    for c in range(NCORES):
        o = res.results[c]["out"].reshape(M, BLOC, L, K, S)
        full[:, c * BLOC:(c + 1) * BLOC] = o
    return full, res


def kernel(sequences, rate_indices, tau_kernel, exchangeability_kernel,
           equilibrium_kernel):
    out, _ = _run(dict(sequences=sequences, rate_indices=rate_indices,
                       tau_kernel=tau_kernel,
                       exchangeability_kernel=exchangeability_kernel,
                       equilibrium_kernel=equilibrium_kernel))
    return out


# revision 30
# speedup vs baseline: 1.8955x; 1.8955x over previous
"""AncProbsLayer Trainium2 kernel (8 NeuronCores, SPMD data-parallel over batch b).

Math: for each (m, b, k):  P = expm(tau[m,b] * Q[m,k])  (20x20 GTR rate matrix),
then anc[m,b,l,k,:] = P[m,b,k, seq[m,b,l], :].

Host does the O(m*k*S^3) eigensolve preprocessing of the 16 tiny 20x20
matrices (R/p/Q/B/eigh -> V, W, lam tables), plus pure index re-encodings
(one-hots of sequences / rate_indices) and softplus of the tiny (m,b)
tau_kernel.  The device computes everything that scales with b/L/k: the tau
gather, e=exp(tau*lam) (ACT), P = (V.e) @ W (PE), the one-hot gather matmul
(PE), and the 21MB output production + DMA.  b is sharded 8 ways.

Per-core layout:
  pairs pr = m*4+q (q = local b), halves h in {0,1} = k groups of 4.
  P-matmul: out[20i, 160kj] += Ve[80(k,s), 20i]^T @ W[80(k,s), 160kj].
  Pairs are packed two-per-tile at 32-partition offsets (PE base-partition
  rule allows bases {0,32,64}); group g = m*2 + q//2, u = q%2.
  Gather matmul (chunk ci): out[128l, 160kj] = oh[20i, 128l]^T @ Ptab,
  where chunk ci covers l = 4*p + ci (stride-4 interleave) so each pair's
  output tile [128, 640] maps to ONE fully-contiguous 320KB DRAM write.
"""

import sys
import numpy as np

for _p in ("/opt/trn_rl_repo", "/root/.axon_site/_ro/trn_rl_repo"):
    if _p not in sys.path:
        sys.path.append(_p)

M, B, L, K, S = 2, 32, 512, 8, 20
NCORES = 8
BLOC = B // NCORES          # 4 b's per core
NPAIR = M * BLOC            # 8 (m, q) pairs per core
NGRP = NPAIR // 2           # 4 groups of 2 pairs
NH = 2                      # k halves
KH = K // NH                # 4 k per half
KD = KH * S                 # 80 = contraction dim per half
KJ = K * S                  # 160 = (k, j) output free dim
NCHUNK = 4                  # l interleave factor
LC = L // NCHUNK            # 128
EPS = 1e-16

# fp16 matmul operands: 1 cycle/row on PE (fp32 = 4) with 10 mantissa bits.
_MM_NP = "float16"

_GRAPH_CACHE = {}


def _softplus(x):
    return np.log1p(np.exp(-np.abs(x))) + np.maximum(x, 0.0)


def _host_prep(sequences, rate_indices, tau_kernel, exchangeability_kernel,
               equilibrium_kernel):
    """Eigensolve preprocessing of the 16 20x20 kernels + input staging."""
    ex = np.asarray(exchangeability_kernel, np.float64)
    eq = np.asarray(equilibrium_kernel, np.float64)
    R = _softplus(0.5 * (ex + np.swapaxes(ex, -1, -2)))          # (m,k,S,S)
    z = eq - eq.max(-1, keepdims=True)
    p = np.exp(z)
    p /= p.sum(-1, keepdims=True)                                # (m,k,S)
    Q = R * p[..., None, :]
    Q = Q - Q.sum(-1, keepdims=True) * np.eye(S)
    mue = -np.sum(p * np.diagonal(Q, axis1=-2, axis2=-1), axis=-1, keepdims=True)
    Q = Q / np.maximum(mue, EPS)[..., None]
    sqrtp = np.sqrt(p)
    Bm = sqrtp[..., :, None] * Q / sqrtp[..., None, :]
    Bm = 0.5 * (Bm + np.swapaxes(Bm, -1, -2))
    lam, U = np.linalg.eigh(Bm)                                  # (m,k,S),(m,k,S,S)
    V = U / sqrtp[..., :, None]                                  # V[m,k,i,s]
    Wm = U * sqrtp[..., :, None]                                 # W[m,k,j,s]

    p_dt = np.dtype(_MM_NP)
    V_in = np.zeros((M * NH, KD, S), p_dt)
    W_in = np.zeros((M * NH, KD, KJ), p_dt)
    lam_in = np.zeros((M * NH, KD, 1), np.float32)
    for m in range(M):
        for h in range(NH):
            mh = m * NH + h
            for kq in range(KH):
                k = h * KH + kq
                r0 = kq * S
                V_in[mh, r0:r0 + S, :] = V[m, k].T.astype(p_dt)      # [s,i]
                W_in[mh, r0:r0 + S, k * S:(k + 1) * S] = Wm[m, k].T.astype(p_dt)
                lam_in[mh, r0:r0 + S, 0] = lam[m, k]

    sp_tauT = np.ascontiguousarray(
        _softplus(np.asarray(tau_kernel, np.float64)).T.astype(np.float32))

    seq = np.asarray(sequences)
    ri = np.asarray(rate_indices)
    in_maps = []
    for c in range(NCORES):
        b0 = c * BLOC
        # one-hot of sequences: oh[g, 32u+i, l] = (seq[m, b0+2*(g%2)+u, l]==i)
        oh = np.zeros((NGRP, 64, L), p_dt)
        for g in range(NGRP):
            m, qh = g // 2, g % 2
            for u in range(2):
                sq = seq[m, b0 + 2 * qh + u]
                oh[g, 32 * u + sq, np.arange(L)] = 1.0
        # one-hot of rate_indices: rioh[m, i, q] = (ri[m, b0+q]==i)
        rioh = np.zeros((M, B, BLOC), np.float32)
        for m in range(M):
            for q in range(BLOC):
                rioh[m, ri[m, b0 + q], q] = 1.0
        in_maps.append({
            "oh": oh,
            "rioh": rioh,
            "sptauT": sp_tauT,
            "V": V_in,
            "W": W_in,
            "lam": lam_in,
        })
    return in_maps


def _build_graph():
    if "nc" in _GRAPH_CACHE:
        return _GRAPH_CACHE["nc"]
    from contextlib import ExitStack
    import concourse.mybir as mybir
    import concourse.tile as tile
    from concourse import bacc

    f32 = mybir.dt.float32
    mm_dt = getattr(mybir.dt, _MM_NP)
    AF = mybir.ActivationFunctionType
    ALU = mybir.AluOpType

    nc = bacc.Bacc("TRN2", target_bir_lowering=False, debug=False,
                   enable_asserts=False)
    oh_e = nc.declare_dram_parameter("oh", [NGRP, 64, L], mm_dt, isOutput=False)
    rioh_e = nc.declare_dram_parameter("rioh", [M, B, BLOC], f32, isOutput=False)
    spt_e = nc.declare_dram_parameter("sptauT", [B, M], f32, isOutput=False)
    V_e = nc.declare_dram_parameter("V", [M * NH, KD, S], mm_dt, isOutput=False)
    W_e = nc.declare_dram_parameter("W", [M * NH, KD, KJ], mm_dt, isOutput=False)
    lam_e = nc.declare_dram_parameter("lam", [M * NH, KD, 1], f32, isOutput=False)
    out_e = nc.declare_dram_parameter("out", [NPAIR, LC, NCHUNK * KJ], f32,
                                      isOutput=True)

    with tile.TileContext(nc) as tc, ExitStack() as ctx:
        const = ctx.enter_context(tc.tile_pool(name="const", bufs=1))
        work = ctx.enter_context(tc.tile_pool(name="work", bufs=3))
        outp = ctx.enter_context(tc.tile_pool(name="outp", bufs=4))
        ps_s = ctx.enter_context(tc.tile_pool(name="ps_s", bufs=1, space="PSUM"))
        ps_p = ctx.enter_context(tc.tile_pool(name="ps_p", bufs=2, space="PSUM"))
        ps_g = ctx.enter_context(tc.tile_pool(name="ps_g", bufs=4, space="PSUM"))

        # ---- static loads + constants
        sptT = const.tile([B, M], f32, tag="sptT")
        nc.sync.dma_start(sptT[:], spt_e[:])
        ones80 = const.tile([1, KD], f32, tag="ones80")
        nc.vector.memset(ones80[:], 1.0)

        V_t, W_t, lam_t = [], [], []
        for mh in range(M * NH):
            vt = const.tile([KD, S], mm_dt, tag=f"V{mh}", name=f"V{mh}")
            nc.sync.dma_start(vt[:], V_e[mh])
            wt = const.tile([KD, KJ], mm_dt, tag=f"W{mh}", name=f"W{mh}")
            nc.sync.dma_start(wt[:], W_e[mh])
            lt = const.tile([KD, 1], f32, tag=f"lam{mh}", name=f"lam{mh}")
            nc.sync.dma_start(lt[:], lam_e[mh])
            V_t.append(vt); W_t.append(wt); lam_t.append(lt)

        oh_t = []
        for g in range(NGRP):
            oh = const.tile([64, L], mm_dt, tag=f"oh{g}", name=f"oh{g}")
            nc.sync.dma_start(oh[:], oh_e[g])
            oh_t.append(oh)

        # ---- tau gather (PE) -> e = exp(tau*lam) (ACT), per m / half
        e_t = []
        for m in range(M):
            rioh = work.tile([B, BLOC], f32, tag="rioh", name="rioh")
            nc.sync.dma_start(rioh[:], rioh_e[m])
            tau_ps = ps_s.tile([1, BLOC], f32, tag="tau_ps", name="tau_ps")
            nc.tensor.matmul(tau_ps[:], sptT[:, m:m + 1], rioh[:])
            tau_sb = work.tile([1, BLOC], f32, tag="tau_sb", name="tau_sb")
            nc.scalar.activation(tau_sb[:], tau_ps[:], AF.Copy)
            taub_ps = ps_s.tile([KD, BLOC], f32, tag="taub_ps", name="taub_ps")
            nc.tensor.matmul(taub_ps[:], ones80[:], tau_sb[:])
            for h in range(NH):
                mh = m * NH + h
                et = const.tile([KD, BLOC], f32, tag=f"e{mh}", name=f"e{mh}")
                nc.scalar.activation(et[:], taub_ps[:], AF.Exp,
                                     scale=lam_t[mh][:])
                e_t.append(et)

        # ---- Ve_all[mh][80, (q,i)] = V[80, i] * e[80, q]  (one DVE op each)
        ve_t = []
        for mh in range(M * NH):
            ve = const.tile([KD, BLOC, S], mm_dt, tag=f"ve{mh}", name=f"ve{mh}")
            v_b = V_t[mh][:].unsqueeze(1).broadcast_to((KD, BLOC, S))
            e_b = e_t[mh][:].unsqueeze(2).broadcast_to((KD, BLOC, S))
            nc.vector.tensor_tensor(ve[:], v_b, e_b, ALU.mult)
            ve_t.append(ve)

        # ---- per group: P construction; per pair: gather + store
        for g in range(NGRP):
            m, qh = g // 2, g % 2
            P_ps = ps_p.tile([64, KJ], f32, tag="P_ps", name=f"P_ps{g}")
            ptab = work.tile([64, KJ], mm_dt, tag="ptab", name=f"ptab{g}")
            for u in range(2):
                q = 2 * qh + u
                pslice = slice(32 * u, 32 * u + S)
                for h in range(NH):
                    mh = m * NH + h
                    nc.tensor.matmul(P_ps[pslice, :],
                                     ve_t[mh][:, q, :], W_t[mh][:],
                                     start=(h == 0), stop=(h == NH - 1))
            if g % 2 == 0:
                nc.scalar.activation(ptab[:], P_ps[:], AF.Copy)
            else:
                nc.vector.tensor_copy(ptab[:], P_ps[:])
            for u in range(2):
                pr = (2 * g + u)
                pslice = slice(32 * u, 32 * u + S)
                o_sb = outp.tile([LC, NCHUNK * KJ], f32, tag="o_sb",
                                 name="o_sb")
                for cj in range(2):
                    g_ps = ps_g.tile([LC, 2 * KJ], f32, tag="g_ps",
                                     name="g_ps")
                    for w in range(2):
                        ci = 2 * cj + w
                        nc.tensor.matmul(
                            g_ps[:, w * KJ:(w + 1) * KJ],
                            oh_t[g][pslice, ci::NCHUNK],
                            ptab[pslice, :])
                    if (u + cj) % 2 == 0:
                        nc.vector.tensor_copy(
                            o_sb[:, 2 * KJ * cj:2 * KJ * (cj + 1)], g_ps[:])
                    else:
                        nc.scalar.activation(
                            o_sb[:, 2 * KJ * cj:2 * KJ * (cj + 1)], g_ps[:],
                            AF.Copy)
                eng = nc.sync if u == 0 else nc.scalar
                eng.dma_start(out_e[pr], o_sb[:])

    nc.compile()
    _GRAPH_CACHE["nc"] = nc
    return nc


def _run(inputs, trace=False):
    from concourse.bass_utils import run_bass_kernel_spmd
    in_maps = _host_prep(**inputs)
    nc = _build_graph()
    res = run_bass_kernel_spmd(nc, in_maps, core_ids=list(range(NCORES)),
                               trace=trace)
    full = np.empty((M, B, L, K, S), np.float32)
    for c in range(NCORES):
        # out[pr, p, (ci, k, s)] holds l = 4*p + ci
        o = res.results[c]["out"].reshape(M, BLOC, LC, NCHUNK, K, S)
        o = np.transpose(o, (0, 1, 2, 3, 4, 5)).reshape(M, BLOC, L, K, S)
        full[:, c * BLOC:(c + 1) * BLOC] = o
    return full, res


def kernel(sequences, rate_indices, tau_kernel, exchangeability_kernel,
           equilibrium_kernel):
    out, _ = _run(dict(sequences=sequences, rate_indices=rate_indices,
                       tau_kernel=tau_kernel,
                       exchangeability_kernel=exchangeability_kernel,
                       equilibrium_kernel=equilibrium_kernel))
    return out


# revision 32
# speedup vs baseline: 2.5970x; 1.3701x over previous
"""AncProbsLayer Trainium2 kernel (8 NeuronCores, SPMD data-parallel over batch b).

Math: for each (m, b, k):  P = expm(tau[m,b] * Q[m,k])  (20x20 GTR rate matrix),
then anc[m,b,l,k,:] = P[m,b,k, seq[m,b,l], :].

Host does the O(m*k*S^3) eigensolve preprocessing of the 16 tiny 20x20
matrices (R/p/Q/B/eigh -> V, W, lam tables), plus pure index re-encodings
(one-hots of sequences / rate_indices) and softplus of the tiny (m,b)
tau_kernel.  The device computes everything that scales with b/L/k: the tau
gather, e=exp(tau*lam) (ACT), P = (V.e) @ W (PE), the one-hot gather matmul
(PE), and the 21MB output production + DMA.  b is sharded 8 ways.

Per-core layout:
  pairs pr = m*4+q (q = local b), halves h in {0,1} = k groups of 4.
  P-matmul: out[20i, 160kj] += Ve[80(k,s), 20i]^T @ W[80(k,s), 160kj].
  Pairs are packed two-per-tile at 32-partition offsets (PE base-partition
  rule allows bases {0,32,64}); group g = m*2 + q//2, u = q%2.
  Gather matmul (chunk ci): out[128l, 160kj] = oh[20i, 128l]^T @ Ptab,
  where chunk ci covers l = 4*p + ci (stride-4 interleave) so each pair's
  [128, 640] output tile maps to a fully-contiguous 320KB DRAM region.

DMA-instruction count is the scarce resource (~626ns serialized HWDGE per
DMA): all constants ride 5 packed input DMAs; output is 4 DMAs (2 pairs
each).  Gather results land in [128, 1024] two-bank PSUM tiles (quadrants
at 0/160/512/672 * 4B) so one strided copy per pair drains them.
"""

import sys
import numpy as np

for _p in ("/opt/trn_rl_repo", "/root/.axon_site/_ro/trn_rl_repo"):
    if _p not in sys.path:
        sys.path.append(_p)

M, B, L, K, S = 2, 32, 512, 8, 20
NCORES = 8
BLOC = B // NCORES          # 4 b's per core
NPAIR = M * BLOC            # 8 (m, q) pairs per core
NGRP = NPAIR // 2           # 4 groups of 2 pairs
NH = 2                      # k halves
KH = K // NH                # 4 k per half
KD = KH * S                 # 80 = contraction dim per half
KJ = K * S                  # 160 = (k, j) output free dim
NCHUNK = 4                  # l interleave factor
LC = L // NCHUNK            # 128
EPS = 1e-16

# fp16 matmul operands: 1 cycle/row on PE (fp32 = 4) with 10 mantissa bits.
_MM_NP = "float16"

_GRAPH_CACHE = {}


def _softplus(x):
    return np.log1p(np.exp(-np.abs(x))) + np.maximum(x, 0.0)


def _host_prep(sequences, rate_indices, tau_kernel, exchangeability_kernel,
               equilibrium_kernel):
    """Eigensolve preprocessing of the 16 20x20 kernels + input staging."""
    ex = np.asarray(exchangeability_kernel, np.float64)
    eq = np.asarray(equilibrium_kernel, np.float64)
    R = _softplus(0.5 * (ex + np.swapaxes(ex, -1, -2)))          # (m,k,S,S)
    z = eq - eq.max(-1, keepdims=True)
    p = np.exp(z)
    p /= p.sum(-1, keepdims=True)                                # (m,k,S)
    Q = R * p[..., None, :]
    Q = Q - Q.sum(-1, keepdims=True) * np.eye(S)
    mue = -np.sum(p * np.diagonal(Q, axis1=-2, axis2=-1), axis=-1, keepdims=True)
    Q = Q / np.maximum(mue, EPS)[..., None]
    sqrtp = np.sqrt(p)
    Bm = sqrtp[..., :, None] * Q / sqrtp[..., None, :]
    Bm = 0.5 * (Bm + np.swapaxes(Bm, -1, -2))
    lam, U = np.linalg.eigh(Bm)                                  # (m,k,S),(m,k,S,S)
    V = U / sqrtp[..., :, None]                                  # V[m,k,i,s]
    Wm = U * sqrtp[..., :, None]                                 # W[m,k,j,s]

    p_dt = np.dtype(_MM_NP)
    # packed per-(m,h) tables, all on KD=80 partitions -> one DMA each
    V_all = np.zeros((KD, M * NH, S), p_dt)
    W_all = np.zeros((KD, M * NH, KJ), p_dt)
    lam_all = np.zeros((KD, M * NH), np.float32)
    for m in range(M):
        for h in range(NH):
            mh = m * NH + h
            for kq in range(KH):
                k = h * KH + kq
                r0 = kq * S
                V_all[r0:r0 + S, mh, :] = V[m, k].T.astype(p_dt)     # [s,i]
                W_all[r0:r0 + S, mh, k * S:(k + 1) * S] = Wm[m, k].T.astype(p_dt)
                lam_all[r0:r0 + S, mh] = lam[m, k]

    sp_tauT = _softplus(np.asarray(tau_kernel, np.float64)).T.astype(np.float32)

    seq = np.asarray(sequences)
    ri = np.asarray(rate_indices)
    in_maps = []
    for c in range(NCORES):
        b0 = c * BLOC
        # one-hot of sequences: oh[32u+i, g, l] = (seq[m, b0+2*(g%2)+u, l]==i)
        oh = np.zeros((64, NGRP, L), p_dt)
        for g in range(NGRP):
            m, qh = g // 2, g % 2
            for u in range(2):
                sq = seq[m, b0 + 2 * qh + u]
                oh[32 * u + sq, g, np.arange(L)] = 1.0
        # aux[:, 0:2] = softplus(tau_kernel)^T; aux[:, 2+m*4+q] = rate one-hot
        aux = np.zeros((B, M + M * BLOC), np.float32)
        aux[:, 0:M] = sp_tauT
        for m in range(M):
            for q in range(BLOC):
                aux[ri[m, b0 + q], M + m * BLOC + q] = 1.0
        in_maps.append({
            "oh": oh,
            "aux": aux,
            "V": V_all,
            "W": W_all,
            "lam": lam_all,
        })
    return in_maps


def _build_graph():
    if "nc" in _GRAPH_CACHE:
        return _GRAPH_CACHE["nc"]
    from contextlib import ExitStack
    import concourse.mybir as mybir
    import concourse.tile as tile
    from concourse import bacc

    f32 = mybir.dt.float32
    mm_dt = getattr(mybir.dt, _MM_NP)
    AF = mybir.ActivationFunctionType
    ALU = mybir.AluOpType

    nc = bacc.Bacc("TRN2", target_bir_lowering=False, debug=False,
                   enable_asserts=False)
    oh_e = nc.declare_dram_parameter("oh", [64, NGRP, L], mm_dt, isOutput=False)
    aux_e = nc.declare_dram_parameter("aux", [B, M + M * BLOC], f32,
                                      isOutput=False)
    V_e = nc.declare_dram_parameter("V", [KD, M * NH, S], mm_dt, isOutput=False)
    W_e = nc.declare_dram_parameter("W", [KD, M * NH, KJ], mm_dt, isOutput=False)
    lam_e = nc.declare_dram_parameter("lam", [KD, M * NH], f32, isOutput=False)
    out_e = nc.declare_dram_parameter("out", [NPAIR, LC, NCHUNK * KJ], f32,
                                      isOutput=True)

    with tile.TileContext(nc) as tc, ExitStack() as ctx:
        const = ctx.enter_context(tc.tile_pool(name="const", bufs=1))
        work = ctx.enter_context(tc.tile_pool(name="work", bufs=3))
        outp = ctx.enter_context(tc.tile_pool(name="outp", bufs=2))
        ps_s = ctx.enter_context(tc.tile_pool(name="ps_s", bufs=1, space="PSUM"))
        ps_p = ctx.enter_context(tc.tile_pool(name="ps_p", bufs=2, space="PSUM"))
        ps_g = ctx.enter_context(tc.tile_pool(name="ps_g", bufs=2, space="PSUM"))

        # ---- packed input DMAs (5)
        aux = const.tile([B, M + M * BLOC], f32, tag="aux")
        nc.sync.dma_start(aux[:], aux_e[:])
        V_a = const.tile([KD, M * NH, S], mm_dt, tag="V_a")
        nc.sync.dma_start(V_a[:], V_e[:])
        W_a = const.tile([KD, M * NH, KJ], mm_dt, tag="W_a")
        nc.scalar.dma_start(W_a[:], W_e[:])
        lam_a = const.tile([KD, M * NH], f32, tag="lam_a")
        nc.sync.dma_start(lam_a[:], lam_e[:])
        oh_a = const.tile([64, NGRP, L], mm_dt, tag="oh_a")
        nc.scalar.dma_start(oh_a[:], oh_e[:])

        ones80 = const.tile([1, KD], f32, tag="ones80")
        nc.vector.memset(ones80[:], 1.0)

        # ---- tau gather (PE) -> e = exp(tau*lam) (ACT), per m / half
        e_t = []
        for m in range(M):
            tau_ps = ps_s.tile([1, BLOC], f32, tag="tau_ps", name="tau_ps")
            nc.tensor.matmul(tau_ps[:], aux[:, m:m + 1],
                             aux[:, M + m * BLOC:M + (m + 1) * BLOC])
            tau_sb = work.tile([1, BLOC], f32, tag="tau_sb", name="tau_sb")
            nc.scalar.activation(tau_sb[:], tau_ps[:], AF.Copy)
            taub_ps = ps_s.tile([KD, BLOC], f32, tag="taub_ps", name="taub_ps")
            nc.tensor.matmul(taub_ps[:], ones80[:], tau_sb[:])
            for h in range(NH):
                mh = m * NH + h
                et = const.tile([KD, BLOC], f32, tag=f"e{mh}", name=f"e{mh}")
                nc.scalar.activation(et[:], taub_ps[:], AF.Exp,
                                     scale=lam_a[:, mh:mh + 1])
                e_t.append(et)

        # ---- Ve_all[mh][80, (q,i)] = V[80, i] * e[80, q]  (one DVE op each)
        ve_t = []
        for mh in range(M * NH):
            ve = const.tile([KD, BLOC, S], mm_dt, tag=f"ve{mh}", name=f"ve{mh}")
            v_b = V_a[:, mh, :].unsqueeze(1).broadcast_to((KD, BLOC, S))
            e_b = e_t[mh][:].unsqueeze(2).broadcast_to((KD, BLOC, S))
            nc.vector.tensor_tensor(ve[:], v_b, e_b, ALU.mult)
            ve_t.append(ve)

        # ---- per group: P construction + gather; one output DMA per 2 pairs
        for g in range(NGRP):
            m, qh = g // 2, g % 2
            P_ps = ps_p.tile([64, KJ], f32, tag="P_ps", name=f"P_ps{g}")
            ptab = work.tile([64, KJ], mm_dt, tag="ptab", name=f"ptab{g}")
            for u in range(2):
                q = 2 * qh + u
                pslice = slice(32 * u, 32 * u + S)
                for h in range(NH):
                    mh = m * NH + h
                    nc.tensor.matmul(P_ps[pslice, :],
                                     ve_t[mh][:, q, :], W_a[:, mh, :],
                                     start=(h == 0), stop=(h == NH - 1))
            if g % 2 == 0:
                nc.scalar.activation(ptab[:], P_ps[:], AF.Copy)
            else:
                nc.vector.tensor_copy(ptab[:], P_ps[:])
            o_sb = outp.tile([LC, 2, NCHUNK * KJ], f32, tag="o_sb",
                             name="o_sb")
            for u in range(2):
                pslice = slice(32 * u, 32 * u + S)
                # [128, 1024] = 2 PSUM banks; chunk ci=(cj,w) at 512*cj+160*w
                g_ps = ps_g.tile([LC, 2, 512], f32, tag="g_ps", name="g_ps")
                for cj in range(2):
                    for w in range(2):
                        ci = 2 * cj + w
                        nc.tensor.matmul(
                            g_ps[:, cj, w * KJ:(w + 1) * KJ],
                            oh_a[pslice, g, ci::NCHUNK],
                            ptab[pslice, :])
                src = g_ps[:, :, 0:2 * KJ]          # [128, 2, 320]
                dst = o_sb[:, u, :].rearrange("p (c r) -> p c r", c=2)
                if (g + u) % 2 == 0:
                    nc.vector.tensor_copy(dst, src)
                else:
                    nc.scalar.activation(dst, src, AF.Copy)
            eng = nc.sync if g % 2 == 0 else nc.scalar
            eng.dma_start(
                out_e[2 * g:2 * g + 2].rearrange("r p f -> p r f"),
                o_sb[:])

    nc.compile()
    _GRAPH_CACHE["nc"] = nc
    return nc


def _run(inputs, trace=False):
    from concourse.bass_utils import run_bass_kernel_spmd
    in_maps = _host_prep(**inputs)
    nc = _build_graph()
    res = run_bass_kernel_spmd(nc, in_maps, core_ids=list(range(NCORES)),
                               trace=trace)
    full = np.empty((M, B, L, K, S), np.float32)
    for c in range(NCORES):
        # out[pr, p, (ci, k, s)] holds l = 4*p + ci
        o = res.results[c]["out"].reshape(M, BLOC, LC, NCHUNK, K, S)
        full[:, c * BLOC:(c + 1) * BLOC] = o.reshape(M, BLOC, L, K, S)
    return full, res


def kernel(sequences, rate_indices, tau_kernel, exchangeability_kernel,
           equilibrium_kernel):
    out, _ = _run(dict(sequences=sequences, rate_indices=rate_indices,
                       tau_kernel=tau_kernel,
                       exchangeability_kernel=exchangeability_kernel,
                       equilibrium_kernel=equilibrium_kernel))
    return out


# revision 35
# speedup vs baseline: 2.6689x; 1.0277x over previous
"""AncProbsLayer Trainium2 kernel (8 NeuronCores, SPMD data-parallel over batch b).

Math: for each (m, b, k):  P = expm(tau[m,b] * Q[m,k])  (20x20 GTR rate matrix),
then anc[m,b,l,k,:] = P[m,b,k, seq[m,b,l], :].

Host does the O(m*k*S^3) eigensolve preprocessing of the 16 tiny 20x20
matrices (R/p/Q/B/eigh -> V, W, lam tables), plus pure index re-encodings
(one-hots of sequences / rate_indices) and softplus of the tiny (m,b)
tau_kernel.  The device computes everything that scales with b/L/k: the tau
gather, e=exp(tau*lam) (ACT), P = (V.e) @ W (PE), the one-hot gather matmul
(PE), and the 21MB output production + DMA.  b is sharded 8 ways.

Per-core layout:
  pairs pr = m*4+q (q = local b), halves h in {0,1} = k groups of 4.
  P-matmul: out[20i, 160kj] += Ve[80(k,s), 20i]^T @ W[80(k,s), 160kj].
  Pairs are packed two-per-tile at 32-partition offsets (PE base-partition
  rule allows bases {0,32,64}); group g = m*2 + q//2, u = q%2.
  Gather matmul (chunk ci): out[128l, 160kj] = oh[20i, 128l]^T @ Ptab,
  where chunk ci covers l = 4*p + ci (stride-4 interleave) so each pair's
  [128, 640] output tile maps to a fully-contiguous 320KB DRAM region.

DMA-instruction count is the scarce resource (~626ns serialized HWDGE per
DMA): all constants ride 5 packed input DMAs; output is 4 DMAs (2 pairs
each).  Gather results land in [128, 1024] two-bank PSUM tiles (quadrants
at 0/160/512/672 * 4B) so one strided copy per pair drains them.
"""

import sys
import numpy as np

for _p in ("/opt/trn_rl_repo", "/root/.axon_site/_ro/trn_rl_repo"):
    if _p not in sys.path:
        sys.path.append(_p)

M, B, L, K, S = 2, 32, 512, 8, 20
NCORES = 8
BLOC = B // NCORES          # 4 b's per core
NPAIR = M * BLOC            # 8 (m, q) pairs per core
NGRP = NPAIR // 2           # 4 groups of 2 pairs
NH = 2                      # k halves
KH = K // NH                # 4 k per half
KD = KH * S                 # 80 = contraction dim per half
KJ = K * S                  # 160 = (k, j) output free dim
NCHUNK = 4                  # l interleave factor
LC = L // NCHUNK            # 128
EPS = 1e-16

# fp16 matmul operands: 1 cycle/row on PE (fp32 = 4) with 10 mantissa bits.
_MM_NP = "float16"

_GRAPH_CACHE = {}


def _softplus(x):
    return np.log1p(np.exp(-np.abs(x))) + np.maximum(x, 0.0)


def _host_prep(sequences, rate_indices, tau_kernel, exchangeability_kernel,
               equilibrium_kernel):
    """Eigensolve preprocessing of the 16 20x20 kernels + input staging."""
    ex = np.asarray(exchangeability_kernel, np.float64)
    eq = np.asarray(equilibrium_kernel, np.float64)
    R = _softplus(0.5 * (ex + np.swapaxes(ex, -1, -2)))          # (m,k,S,S)
    z = eq - eq.max(-1, keepdims=True)
    p = np.exp(z)
    p /= p.sum(-1, keepdims=True)                                # (m,k,S)
    Q = R * p[..., None, :]
    Q = Q - Q.sum(-1, keepdims=True) * np.eye(S)
    mue = -np.sum(p * np.diagonal(Q, axis1=-2, axis2=-1), axis=-1, keepdims=True)
    Q = Q / np.maximum(mue, EPS)[..., None]
    sqrtp = np.sqrt(p)
    Bm = sqrtp[..., :, None] * Q / sqrtp[..., None, :]
    Bm = 0.5 * (Bm + np.swapaxes(Bm, -1, -2))
    lam, U = np.linalg.eigh(Bm)                                  # (m,k,S),(m,k,S,S)
    V = U / sqrtp[..., :, None]                                  # V[m,k,i,s]
    Wm = U * sqrtp[..., :, None]                                 # W[m,k,j,s]

    p_dt = np.dtype(_MM_NP)
    # packed per-(m,h) tables, all on KD=80 partitions -> one DMA each
    V_all = np.zeros((KD, M * NH, S), p_dt)
    W_all = np.zeros((KD, M * NH, KJ), p_dt)
    lam_all = np.zeros((KD, M * NH), np.float32)
    for m in range(M):
        for h in range(NH):
            mh = m * NH + h
            for kq in range(KH):
                k = h * KH + kq
                r0 = kq * S
                V_all[r0:r0 + S, mh, :] = V[m, k].T.astype(p_dt)     # [s,i]
                W_all[r0:r0 + S, mh, k * S:(k + 1) * S] = Wm[m, k].T.astype(p_dt)
                lam_all[r0:r0 + S, mh] = lam[m, k]

    sp_tauT = _softplus(np.asarray(tau_kernel, np.float64)).T.astype(np.float32)

    seq = np.asarray(sequences)
    ri = np.asarray(rate_indices)
    in_maps = []
    for c in range(NCORES):
        b0 = c * BLOC
        # one-hot of sequences: oh[32u+i, g, l] = (seq[m, b0+2*(g%2)+u, l]==i)
        oh = np.zeros((64, NGRP, L), p_dt)
        for g in range(NGRP):
            m, qh = g // 2, g % 2
            for u in range(2):
                sq = seq[m, b0 + 2 * qh + u]
                oh[32 * u + sq, g, np.arange(L)] = 1.0
        # aux[:, 0:2] = softplus(tau_kernel)^T; aux[:, 2+m*4+q] = rate one-hot
        aux = np.zeros((B, M + M * BLOC), np.float32)
        aux[:, 0:M] = sp_tauT
        for m in range(M):
            for q in range(BLOC):
                aux[ri[m, b0 + q], M + m * BLOC + q] = 1.0
        in_maps.append({
            "oh": oh,
            "aux": aux,
            "V": V_all,
            "W": W_all,
            "lam": lam_all,
        })
    return in_maps


def _build_graph():
    if "nc" in _GRAPH_CACHE:
        return _GRAPH_CACHE["nc"]
    from contextlib import ExitStack
    import concourse.mybir as mybir
    import concourse.tile as tile
    from concourse import bacc

    f32 = mybir.dt.float32
    mm_dt = getattr(mybir.dt, _MM_NP)
    AF = mybir.ActivationFunctionType
    ALU = mybir.AluOpType

    nc = bacc.Bacc("TRN2", target_bir_lowering=False, debug=False,
                   enable_asserts=False)
    oh_e = nc.declare_dram_parameter("oh", [64, NGRP, L], mm_dt, isOutput=False)
    aux_e = nc.declare_dram_parameter("aux", [B, M + M * BLOC], f32,
                                      isOutput=False)
    V_e = nc.declare_dram_parameter("V", [KD, M * NH, S], mm_dt, isOutput=False)
    W_e = nc.declare_dram_parameter("W", [KD, M * NH, KJ], mm_dt, isOutput=False)
    lam_e = nc.declare_dram_parameter("lam", [KD, M * NH], f32, isOutput=False)
    out_e = nc.declare_dram_parameter("out", [NPAIR, LC, NCHUNK * KJ], f32,
                                      isOutput=True)

    with tile.TileContext(nc) as tc, ExitStack() as ctx:
        const = ctx.enter_context(tc.tile_pool(name="const", bufs=1))
        work = ctx.enter_context(tc.tile_pool(name="work", bufs=3))
        outp = ctx.enter_context(tc.tile_pool(name="outp", bufs=2))
        ps_s = ctx.enter_context(tc.tile_pool(name="ps_s", bufs=1, space="PSUM"))
        ps_p = ctx.enter_context(tc.tile_pool(name="ps_p", bufs=1, space="PSUM"))
        ps_g = ctx.enter_context(tc.tile_pool(name="ps_g", bufs=3, space="PSUM"))

        # ---- packed input DMAs (5), critical-path order: aux/lam feed the
        # tau->e chain, then V (ve), W (P), oh (gather).
        aux = const.tile([B, M + M * BLOC], f32, tag="aux")
        nc.sync.dma_start(aux[:], aux_e[:])
        lam_a = const.tile([KD, M * NH], f32, tag="lam_a")
        nc.sync.dma_start(lam_a[:], lam_e[:])
        V_a = const.tile([KD, M * NH, S], mm_dt, tag="V_a")
        nc.sync.dma_start(V_a[:], V_e[:])
        W_a = const.tile([KD, M * NH, KJ], mm_dt, tag="W_a")
        nc.scalar.dma_start(W_a[:], W_e[:])
        oh_a = const.tile([64, NGRP, L], mm_dt, tag="oh_a")
        nc.scalar.dma_start(oh_a[:], oh_e[:])

        ones80 = const.tile([1, KD], f32, tag="ones80")
        nc.vector.memset(ones80[:], 1.0)
        # Dummy Exp with no input deps: hoists the ACT table load (~1.3us)
        # off the critical path, overlapping it with the input DMAs.
        warm = work.tile([1, KD], f32, tag="warm", name="warm")
        nc.scalar.activation(warm[:], ones80[:], AF.Exp)

        # ---- tau gather (PE) -> e = exp(tau*lam) (ACT), per m / half
        e_t = []
        for m in range(M):
            tau_ps = ps_s.tile([1, BLOC], f32, tag="smallps", name="tau_ps")
            nc.tensor.matmul(tau_ps[:], aux[:, m:m + 1],
                             aux[:, M + m * BLOC:M + (m + 1) * BLOC])
            tau_sb = work.tile([1, BLOC], f32, tag="tau_sb", name="tau_sb")
            nc.scalar.activation(tau_sb[:], tau_ps[:], AF.Copy)
            taub_ps = ps_s.tile([KD, BLOC], f32, tag="smallps", name="taub_ps")
            nc.tensor.matmul(taub_ps[:], ones80[:], tau_sb[:])
            for h in range(NH):
                mh = m * NH + h
                et = const.tile([KD, BLOC], f32, tag=f"e{mh}", name=f"e{mh}")
                nc.scalar.activation(et[:], taub_ps[:], AF.Exp,
                                     scale=lam_a[:, mh:mh + 1])
                e_t.append(et)

        # ---- Ve_all[mh][80, (q,i)] = V[80, i] * e[80, q]  (one DVE op each)
        ve_t = []
        for mh in range(M * NH):
            ve = const.tile([KD, BLOC, S], mm_dt, tag=f"ve{mh}", name=f"ve{mh}")
            v_b = V_a[:, mh, :].unsqueeze(1).broadcast_to((KD, BLOC, S))
            e_b = e_t[mh][:].unsqueeze(2).broadcast_to((KD, BLOC, S))
            nc.vector.tensor_tensor(ve[:], v_b, e_b, ALU.mult)
            ve_t.append(ve)

        # ---- per group: P construction + gather; one output DMA per 2 pairs
        for g in range(NGRP):
            m, qh = g // 2, g % 2
            P_ps = ps_p.tile([64, KJ], f32, tag="P_ps", name=f"P_ps{g}")
            ptab = work.tile([64, KJ], mm_dt, tag="ptab", name=f"ptab{g}")
            for u in range(2):
                q = 2 * qh + u
                pslice = slice(32 * u, 32 * u + S)
                for h in range(NH):
                    mh = m * NH + h
                    nc.tensor.matmul(P_ps[pslice, :],
                                     ve_t[mh][:, q, :], W_a[:, mh, :],
                                     start=(h == 0), stop=(h == NH - 1))
            if g % 2 == 0:
                nc.scalar.activation(ptab[:], P_ps[:], AF.Copy)
            else:
                nc.vector.tensor_copy(ptab[:], P_ps[:])
            o_sb = outp.tile([LC, 2, NCHUNK * KJ], f32, tag="o_sb",
                             name="o_sb")
            for u in range(2):
                pslice = slice(32 * u, 32 * u + S)
                # [128, 1024] = 2 PSUM banks; chunk ci=(cj,w) at 512*cj+160*w
                g_ps = ps_g.tile([LC, 2, 512], f32, tag="g_ps", name="g_ps")
                for cj in range(2):
                    for w in range(2):
                        ci = 2 * cj + w
                        nc.tensor.matmul(
                            g_ps[:, cj, w * KJ:(w + 1) * KJ],
                            oh_a[pslice, g, ci::NCHUNK],
                            ptab[pslice, :])
                src = g_ps[:, :, 0:2 * KJ]          # [128, 2, 320]
                dst = o_sb[:, u, :].rearrange("p (c r) -> p c r", c=2)
                if (g + u) % 2 == 0:
                    nc.vector.tensor_copy(dst, src)
                else:
                    nc.scalar.activation(dst, src, AF.Copy)
            eng = nc.sync if g % 2 == 0 else nc.scalar
            eng.dma_start(
                out_e[2 * g:2 * g + 2].rearrange("r p f -> p r f"),
                o_sb[:])

    nc.compile()
    _GRAPH_CACHE["nc"] = nc
    return nc


def _run(inputs, trace=False):
    from concourse.bass_utils import run_bass_kernel_spmd
    in_maps = _host_prep(**inputs)
    nc = _build_graph()
    res = run_bass_kernel_spmd(nc, in_maps, core_ids=list(range(NCORES)),
                               trace=trace)
    full = np.empty((M, B, L, K, S), np.float32)
    for c in range(NCORES):
        # out[pr, p, (ci, k, s)] holds l = 4*p + ci
        o = res.results[c]["out"].reshape(M, BLOC, LC, NCHUNK, K, S)
        full[:, c * BLOC:(c + 1) * BLOC] = o.reshape(M, BLOC, L, K, S)
    return full, res


def kernel(sequences, rate_indices, tau_kernel, exchangeability_kernel,
           equilibrium_kernel):
    out, _ = _run(dict(sequences=sequences, rate_indices=rate_indices,
                       tau_kernel=tau_kernel,
                       exchangeability_kernel=exchangeability_kernel,
                       equilibrium_kernel=equilibrium_kernel))
    return out


# revision 40
# speedup vs baseline: 2.9577x; 1.1082x over previous
"""AncProbsLayer Trainium2 kernel (8 NeuronCores, SPMD data-parallel over batch b).

Math: for each (m, b, k):  P = expm(tau[m,b] * Q[m,k])  (20x20 GTR rate matrix),
then anc[m,b,l,k,:] = P[m,b,k, seq[m,b,l], :].

Host does the O(m*k*S^3) eigensolve preprocessing of the 16 tiny 20x20
matrices (R/p/Q/B/eigh -> V, W, lam tables), plus pure index re-encodings
(one-hots of sequences / rate_indices) and softplus of the tiny (m,b)
tau_kernel.  The device computes everything that scales with b/L/k: the tau
gather, e=exp(tau*lam) (ACT), P = (V.e) @ W (PE), the one-hot gather matmul
(PE), and the 21MB output production + DMA.  b is sharded 8 ways.

Per-core layout:
  pairs pr = m*4+q (q = local b), halves h in {0,1} = k groups of 4.
  P-matmul: out[20i, 160kj] += Ve[80(k,s), 20i]^T @ W[80(k,s), 160kj].
  Pairs are packed two-per-tile at 32-partition offsets (PE base-partition
  rule allows bases {0,32,64}); group g = m*2 + q//2, u = q%2.
  Gather matmul (chunk ci): out[128l, 160kj] = oh[20i, 128l]^T @ Ptab,
  where chunk ci covers l = 4*p + ci (stride-4 interleave) so each pair's
  [128, 640] output tile maps to a fully-contiguous 320KB DRAM region.

DMA-instruction count is the scarce resource (~626ns serialized HWDGE per
DMA): all constants ride 5 packed input DMAs; output is 4 DMAs (2 pairs
each).  Gather results land in [128, 1024] two-bank PSUM tiles (quadrants
at 0/160/512/672 * 4B) so one strided copy per pair drains them.
"""

import sys
import numpy as np

for _p in ("/opt/trn_rl_repo", "/root/.axon_site/_ro/trn_rl_repo"):
    if _p not in sys.path:
        sys.path.append(_p)

M, B, L, K, S = 2, 32, 512, 8, 20
NCORES = 8
BLOC = B // NCORES          # 4 b's per core
NPAIR = M * BLOC            # 8 (m, q) pairs per core
NGRP = NPAIR // 2           # 4 groups of 2 pairs
NH = 2                      # k halves
KH = K // NH                # 4 k per half
KD = KH * S                 # 80 = contraction dim per half
KJ = K * S                  # 160 = (k, j) output free dim
NCHUNK = 4                  # l interleave factor
LC = L // NCHUNK            # 128
EPS = 1e-16

# fp16 matmul operands: 1 cycle/row on PE (fp32 = 4) with 10 mantissa bits.
_MM_NP = "float16"

_GRAPH_CACHE = {}


def _softplus(x):
    return np.log1p(np.exp(-np.abs(x))) + np.maximum(x, 0.0)


def _host_prep(sequences, rate_indices, tau_kernel, exchangeability_kernel,
               equilibrium_kernel):
    """Eigensolve preprocessing of the 16 20x20 kernels + input staging."""
    ex = np.asarray(exchangeability_kernel, np.float64)
    eq = np.asarray(equilibrium_kernel, np.float64)
    R = _softplus(0.5 * (ex + np.swapaxes(ex, -1, -2)))          # (m,k,S,S)
    z = eq - eq.max(-1, keepdims=True)
    p = np.exp(z)
    p /= p.sum(-1, keepdims=True)                                # (m,k,S)
    Q = R * p[..., None, :]
    Q = Q - Q.sum(-1, keepdims=True) * np.eye(S)
    mue = -np.sum(p * np.diagonal(Q, axis1=-2, axis2=-1), axis=-1, keepdims=True)
    Q = Q / np.maximum(mue, EPS)[..., None]
    sqrtp = np.sqrt(p)
    Bm = sqrtp[..., :, None] * Q / sqrtp[..., None, :]
    Bm = 0.5 * (Bm + np.swapaxes(Bm, -1, -2))
    lam, U = np.linalg.eigh(Bm)                                  # (m,k,S),(m,k,S,S)
    V = U / sqrtp[..., :, None]                                  # V[m,k,i,s]
    Wm = U * sqrtp[..., :, None]                                 # W[m,k,j,s]

    p_dt = np.dtype(_MM_NP)
    # packed per-(m,h) tables, all on KD=80 partitions -> one DMA each
    V_all = np.zeros((KD, M * NH, S), p_dt)
    W_all = np.zeros((KD, M * NH, KJ), p_dt)
    lam_all = np.zeros((KD, M * NH), np.float32)
    for m in range(M):
        for h in range(NH):
            mh = m * NH + h
            for kq in range(KH):
                k = h * KH + kq
                r0 = kq * S
                V_all[r0:r0 + S, mh, :] = V[m, k].T.astype(p_dt)     # [s,i]
                W_all[r0:r0 + S, mh, k * S:(k + 1) * S] = Wm[m, k].T.astype(p_dt)
                lam_all[r0:r0 + S, mh] = lam[m, k]

    sp_tauT = _softplus(np.asarray(tau_kernel, np.float64)).T.astype(np.float32)

    seq = np.asarray(sequences)
    ri = np.asarray(rate_indices)
    in_maps = []
    for c in range(NCORES):
        b0 = c * BLOC
        # one-hot of sequences: oh[32u+i, g, l] = (seq[m, b0+2*(g%2)+u, l]==i)
        oh = np.zeros((64, NGRP, L), p_dt)
        for g in range(NGRP):
            m, qh = g // 2, g % 2
            for u in range(2):
                sq = seq[m, b0 + 2 * qh + u]
                oh[32 * u + sq, g, np.arange(L)] = 1.0
        # aux[:, m*KD:(m+1)*KD] = softplus(tau_kernel)[m] replicated KD wide
        # (so one matmul yields tau broadcast over the 80 contraction rows);
        # aux[:, 2*KD + m*4 + q] = rate one-hot.
        aux = np.zeros((B, M * KD + M * BLOC), np.float32)
        for m in range(M):
            aux[:, m * KD:(m + 1) * KD] = sp_tauT[:, m:m + 1]
            for q in range(BLOC):
                aux[ri[m, b0 + q], M * KD + m * BLOC + q] = 1.0
        in_maps.append({
            "oh": oh,
            "aux": aux,
            "V": V_all,
            "W": W_all,
            "lam": lam_all,
        })
    return in_maps


def _build_graph():
    if "nc" in _GRAPH_CACHE:
        return _GRAPH_CACHE["nc"]
    from contextlib import ExitStack
    import concourse.mybir as mybir
    import concourse.tile as tile
    from concourse import bacc

    f32 = mybir.dt.float32
    mm_dt = getattr(mybir.dt, _MM_NP)
    AF = mybir.ActivationFunctionType
    ALU = mybir.AluOpType

    nc = bacc.Bacc("TRN2", target_bir_lowering=False, debug=False,
                   enable_asserts=False)
    oh_e = nc.declare_dram_parameter("oh", [64, NGRP, L], mm_dt, isOutput=False)
    aux_e = nc.declare_dram_parameter("aux", [B, M * KD + M * BLOC], f32,
                                      isOutput=False)
    V_e = nc.declare_dram_parameter("V", [KD, M * NH, S], mm_dt, isOutput=False)
    W_e = nc.declare_dram_parameter("W", [KD, M * NH, KJ], mm_dt, isOutput=False)
    lam_e = nc.declare_dram_parameter("lam", [KD, M * NH], f32, isOutput=False)
    out_e = nc.declare_dram_parameter("out", [NPAIR, LC, NCHUNK * KJ], f32,
                                      isOutput=True)

    with tile.TileContext(nc) as tc, ExitStack() as ctx:
        const = ctx.enter_context(tc.tile_pool(name="const", bufs=1))
        work = ctx.enter_context(tc.tile_pool(name="work", bufs=3))
        outp = ctx.enter_context(tc.tile_pool(name="outp", bufs=3))
        ps_p = ctx.enter_context(tc.tile_pool(name="ps_p", bufs=2, space="PSUM"))
        ps_g = ctx.enter_context(tc.tile_pool(name="ps_g", bufs=3, space="PSUM"))

        # ---- packed input DMAs, one ring, critical-path order: aux/lam feed
        # the tau->e chain, oh[g0] unblocks the first gathers, then V/W/rest.
        aux = const.tile([B, M * KD + M * BLOC], f32, tag="aux")
        nc.sync.dma_start(aux[:], aux_e[:])
        lam_a = const.tile([KD, M * NH], f32, tag="lam_a")
        nc.sync.dma_start(lam_a[:], lam_e[:])
        V_a = const.tile([KD, M * NH, S], mm_dt, tag="V_a")
        nc.sync.dma_start(V_a[:], V_e[:])
        oh_t = [const.tile([64, L], mm_dt, tag=f"oh{g}", name=f"oh{g}")
                for g in range(NGRP)]
        nc.sync.dma_start(oh_t[0][:], oh_e[:, 0, :])
        W_a = const.tile([KD, M * NH, KJ], mm_dt, tag="W_a")
        nc.sync.dma_start(W_a[:], W_e[:])
        for g in range(1, NGRP):
            nc.sync.dma_start(oh_t[g][:], oh_e[:, g, :])

        ones80 = const.tile([1, KD], f32, tag="ones80")
        nc.vector.memset(ones80[:], 1.0)
        # Dummy Exp with no input deps: hoists the ACT table load (~1.3us)
        # off the critical path, overlapping it with the input DMAs.
        warm = work.tile([1, KD], f32, tag="warm", name="warm")
        nc.scalar.activation(warm[:], ones80[:], AF.Exp)

        # ---- tau gather (PE, one matmul: replicated-tau stationary gives the
        # KD-row broadcast directly) -> e = exp(tau*lam) (ACT), per m / half
        e_t = []
        for m in range(M):
            taub_ps = ps_g.tile([KD, BLOC], f32, tag="g_ps", name=f"taub{m}")
            nc.tensor.matmul(taub_ps[:], aux[:, m * KD:(m + 1) * KD],
                             aux[:, M * KD + m * BLOC:M * KD + (m + 1) * BLOC])
            for h in range(NH):
                mh = m * NH + h
                et = const.tile([KD, BLOC], f32, tag=f"e{mh}", name=f"e{mh}")
                nc.scalar.activation(et[:], taub_ps[:], AF.Exp,
                                     scale=lam_a[:, mh:mh + 1])
                e_t.append(et)

        # ---- Ve_all[mh][80, (q,i)] = V[80, i] * e[80, q]  (one DVE op each)
        ve_t = []
        for mh in range(M * NH):
            ve = const.tile([KD, BLOC, S], mm_dt, tag=f"ve{mh}", name=f"ve{mh}")
            v_b = V_a[:, mh, :].unsqueeze(1).broadcast_to((KD, BLOC, S))
            e_b = e_t[mh][:].unsqueeze(2).broadcast_to((KD, BLOC, S))
            nc.vector.tensor_tensor(ve[:], v_b, e_b, ALU.mult)
            ve_t.append(ve)

        # ---- per group: P construction + gather; one output DMA per 2 pairs
        for g in range(NGRP):
            m, qh = g // 2, g % 2
            P_ps = ps_p.tile([64, KJ], f32, tag="P_ps", name=f"P_ps{g}")
            ptab = work.tile([64, KJ], mm_dt, tag="ptab", name=f"ptab{g}")
            for u in range(2):
                q = 2 * qh + u
                pslice = slice(32 * u, 32 * u + S)
                for h in range(NH):
                    mh = m * NH + h
                    nc.tensor.matmul(P_ps[pslice, :],
                                     ve_t[mh][:, q, :], W_a[:, mh, :],
                                     start=(h == 0), stop=(h == NH - 1))
            if g % 2 == 0:
                nc.scalar.activation(ptab[:], P_ps[:], AF.Copy)
            else:
                nc.vector.tensor_copy(ptab[:], P_ps[:])
            o_sb = outp.tile([LC, 2, NCHUNK * KJ], f32, tag="o_sb",
                             name="o_sb")
            for u in range(2):
                pslice = slice(32 * u, 32 * u + S)
                # [128, 1024] = 2 PSUM banks; chunk ci=(cj,w) at 512*cj+160*w
                g_ps = ps_g.tile([LC, 2, 512], f32, tag="g_ps", name="g_ps")
                for cj in range(2):
                    for w in range(2):
                        ci = 2 * cj + w
                        nc.tensor.matmul(
                            g_ps[:, cj, w * KJ:(w + 1) * KJ],
                            oh_t[g][pslice, ci::NCHUNK],
                            ptab[pslice, :])
                src = g_ps[:, :, 0:2 * KJ]          # [128, 2, 320]
                dst = o_sb[:, u, :].rearrange("p (c r) -> p c r", c=2)
                if (g + u) % 2 == 0:
                    nc.vector.tensor_copy(dst, src)
                else:
                    nc.scalar.activation(dst, src, AF.Copy)
            eng = nc.sync if g % 2 == 0 else nc.scalar
            eng.dma_start(
                out_e[2 * g:2 * g + 2].rearrange("r p f -> p r f"),
                o_sb[:])

    nc.compile()
    _GRAPH_CACHE["nc"] = nc
    return nc


def _run(inputs, trace=False):
    from concourse.bass_utils import run_bass_kernel_spmd
    in_maps = _host_prep(**inputs)
    nc = _build_graph()
    res = run_bass_kernel_spmd(nc, in_maps, core_ids=list(range(NCORES)),
                               trace=trace)
    full = np.empty((M, B, L, K, S), np.float32)
    for c in range(NCORES):
        # out[pr, p, (ci, k, s)] holds l = 4*p + ci
        o = res.results[c]["out"].reshape(M, BLOC, LC, NCHUNK, K, S)
        full[:, c * BLOC:(c + 1) * BLOC] = o.reshape(M, BLOC, L, K, S)
    return full, res


def kernel(sequences, rate_indices, tau_kernel, exchangeability_kernel,
           equilibrium_kernel):
    out, _ = _run(dict(sequences=sequences, rate_indices=rate_indices,
                       tau_kernel=tau_kernel,
                       exchangeability_kernel=exchangeability_kernel,
                       equilibrium_kernel=equilibrium_kernel))
    return out
